# revision 11
# baseline (speedup 1.0000x reference)
"""Bag-of-words classifier kernel for Trainium2 (8 NeuronCores, data-parallel).

Math: logits[b, c] = sum_s [ids[b,s] != 0] * W[c, ids[b,s]] + b[c].

Per core (64 rows, 32768 tokens), the gather runs on the Ant dma_gather
SWDGE instruction with a radix-2 table:
  - table rows (256B stride) hold vocab pairs {2q, 2q+1} as 4 f32:
    [W0[2q], W1[2q], W0[2q+1], W1[2q+1]]; row 0 slots 0:2 zeroed (pad mask).
  - gather index = id >> 1 (max 25128, fits int16), elem_size = 4 f32 (16B).
  - DVE selects the pair by lo = id & 1, reduces over tokens,
    folds partition halves, adds bias.

Token order i <-> (p = i % 128, j = i // 128 within chunk); partition p < 64
holds row p tokens [0,256), p >= 64 holds row p-64 tokens [256,512).

v2: idx>>1 (int16) and lo-bit selection masks are precomputed on host, so
gathers issue as soon as one input DMA lands; all chunks share one
num_idxs register (avoids NX WAR stalls between gather dispatches); one
gather per SWDGE queue, emitted back-to-back.
"""

import os

import numpy as np

import concourse.bass as bass
import concourse.tile as tile
from concourse import bacc, mybir
from concourse.bass_utils import run_bass_kernel_spmd

B, S, V, C = 512, 512, 50257, 2
NCORES = 8
RPC = B // NCORES  # rows per core = 64
P = 128
NI = RPC * S  # gathers per core = 32768
QROWS = (V + 1) // 2 + 1  # 25130 radix-2 table rows (incl. pad row for odd V)

NQUEUES = int(os.environ.get("BOW_NQUEUES", "4"))
SCRATCH = int(os.environ.get("BOW_SCRATCH", "32768"))
# (queue, idxs) per wave; one gather per queue per wave, so wave N's SDMA
# drain overlaps wave N+1's descriptor generation and the final drain tail
# is only the (small) last wave. Queue 0 measures ~3-4us slower per
# instruction than queues 1-3, so it gets smaller chunks.
_WAVE_SPEC = os.environ.get(
    "BOW_WAVESPEC",
    "0:5120,1:5632,2:5632,3:5632;0:1280,1:2176,2:2176,3:2176;0:256,1:896,2:896,3:896",
)
WAVE_CHUNKS = [
    [(int(q), int(n)) for q, n in (c.split(":") for c in w.split(","))]
    for w in _WAVE_SPEC.split(";")
]
CHUNK_QUEUES = [q for w in WAVE_CHUNKS for q, _ in w]
CHUNK_SIZES = [n for w in WAVE_CHUNKS for _, n in w]
assert sum(CHUNK_SIZES) == NI, sum(CHUNK_SIZES)
assert all(n % 128 == 0 for n in CHUNK_SIZES)
NCHUNKS = len(CHUNK_SIZES)

LAST_EXEC_TIME_NS = None

_cache = {}


def _raw_dma_gather(
    nc, out_ap, in_ap, idxs_ap, num_idxs_reg, num_idxs, elem_size, elem_step,
    queue_num=0,
):
    """bass dma_gather without the elem_size*dtype%256 assert (ucode handles
    small elem_size with a 256B-multiple row stride; HW-verified)."""
    from concourse.bass import exact_div

    g = nc.gpsimd
    assert idxs_ap.dtype == mybir.dt.int16
    stride_bytes = elem_step * mybir.dt.size(in_ap.dtype)
    stride_bytes_256 = exact_div(stride_bytes, 256)
    _in_ap = g.lower_ap_dma(in_ap, for_custom_bir_dma=True)
    _idxs_ap = g.lower_ap(idxs_ap)
    _out_ap = g.lower_ap(out_ap)
    return g.add_instruction(
        mybir.InstDMAGatherAnt(
            name=nc.get_next_instruction_name(),
            ins=[*_in_ap, _idxs_ap, g.lower_val_access(num_idxs_reg)],
            outs=[_out_ap],
            transpose=False,
            num_idxs=num_idxs,
            elem_size=elem_size,
            stride_bytes_256=stride_bytes_256,
            gen_mode=0,
            single_packet=False,
            queue_num=queue_num,
            sbuf_tokens_per_rank=0,
            sbuf_free_dim_per_rank=0,
            sbuf_free_dim_pad_per_rank=0,
            sbuf_byte_offset=0,
        )
    )


def _build() -> bass.Bass:
    nc = bacc.Bacc(
        "TRN2",
        target_bir_lowering=False,
        debug=False,
        num_devices=NCORES,
        num_swdge_queues=NQUEUES,
        dynamic_dma_scratch_size=SCRATCH,
    )
    # wrapped+replicated (id >> 1) in gather order, int16, host-prepared
    idx16_d = nc.dram_tensor("idx16", [P, NI // 16], mybir.dt.int16, kind="ExternalInput")
    # lo2[p, j, c] = float(ids[p, j] & 1) replicated over c, host-prepared
    lo2_d = nc.dram_tensor("lo2", [P, NI // P, C], mybir.dt.float32, kind="ExternalInput")
    tab_d = nc.dram_tensor("table", [QROWS, 64], mybir.dt.float32, kind="ExternalInput")
    bias_d = nc.dram_tensor("bias", [RPC, C], mybir.dt.float32, kind="ExternalInput")
    out_d = nc.dram_tensor("out", [RPC, C], mybir.dt.float32, kind="ExternalOutput")

    sizes = CHUNK_SIZES
    offs = [sum(sizes[:k]) for k in range(len(sizes))]  # token offsets (idxs)

    with tile.TileContext(nc) as tc:
        with tc.tile_pool(name="sbuf", bufs=1) as pool:
            lo2 = pool.tile([P, NI // P, C], mybir.dt.float32)
            r = pool.tile([P, C], mybir.dt.float32)
            hi = pool.tile([RPC, C], mybir.dt.float32)
            bias_sb = pool.tile([RPC, C], mybir.dt.float32)
            out_sb = pool.tile([RPC, C], mybir.dt.float32)

            # warmup: a small gather issued first pulls the ~6us GPSIMD
            # ucode IRAM load (MODIFY_POOL_CONFIG) off the critical path
            if int(os.environ.get("BOW_WARMUP", "1")):
                widx = pool.tile([P, 8], mybir.dt.int16)
                wg = pool.tile([P, 1, 4], mybir.dt.float32)
                wjunk = pool.tile([P, 1, 4], mybir.dt.float32)
                nc.gpsimd.memset(widx[:], 0)
                wreg = nc.gpsimd.to_reg(128)
                _raw_dma_gather(nc, wg[:], tab_d[:, 0:4], widx[:], wreg, 128, 4,
                                64, queue_num=1)
                # consume wg promptly so its DMA-sem lane recycles cleanly
                nc.vector.tensor_copy(wjunk[:], wg[:])

            idx16_k = [
                pool.tile([P, n // 16], mybir.dt.int16, name=f"idx16_{k}")
                for k, n in enumerate(sizes)
            ]
            g_k = [
                pool.tile([P, n // P, 4], mybir.dt.float32, name=f"g{k}")
                for k, n in enumerate(sizes)
            ]
            vals_k = [
                pool.tile([P, n // P, C], mybir.dt.float32, name=f"vals{k}")
                for k, n in enumerate(sizes)
            ]
            rk = [
                pool.tile([P, C], mybir.dt.float32, name=f"rk{k}")
                for k in range(len(sizes))
            ]

            # wave-1 idx slices ride Sync, the rest ride Scalar, so the
            # first gathers are not queued behind later-wave input DMAs
            nwave1 = len(WAVE_CHUNKS[0])
            for k, n in enumerate(sizes):
                ws = slice(offs[k] // 16, (offs[k] + n) // 16)
                eng = nc.sync if k < nwave1 else nc.scalar
                eng.dma_start(out=idx16_k[k][:], in_=idx16_d[:, ws])
            nc.scalar.dma_start(out=lo2[:], in_=lo2_d[:])
            nc.scalar.dma_start(out=bias_sb[:], in_=bias_d[:])

            # one register per distinct num_idxs value, shared across chunks
            # (avoids WAR stalls at the NX between gather dispatches)
            regs = {}
            for n in set(sizes):
                regs[n] = nc.gpsimd.to_reg(n)

            for k, n in enumerate(sizes):
                _raw_dma_gather(
                    nc,
                    g_k[k][:],
                    tab_d[:, 0:4],
                    idx16_k[k][:],
                    regs[n],
                    n,
                    4,
                    64,
                    queue_num=CHUNK_QUEUES[k],
                )

            for k, n in enumerate(sizes):
                jc = n // P
                js = slice(offs[k] // P, offs[k] // P + jc)
                # vals = g02 + lo2 * (g24 - g02)
                nc.vector.tensor_tensor(
                    out=vals_k[k][:],
                    in0=g_k[k][:, :, 2:4],
                    in1=g_k[k][:, :, 0:2],
                    op=mybir.AluOpType.subtract,
                )
                nc.vector.tensor_tensor(
                    out=vals_k[k][:],
                    in0=vals_k[k][:],
                    in1=lo2[:, js, :],
                    op=mybir.AluOpType.mult,
                )
                nc.vector.tensor_tensor(
                    out=vals_k[k][:],
                    in0=vals_k[k][:],
                    in1=g_k[k][:, :, 0:2],
                    op=mybir.AluOpType.add,
                )
                # rk[p, c] = sum_j vals[p, j, c]
                nc.vector.tensor_reduce(
                    out=rk[k][:],
                    in_=vals_k[k][:].transpose([0, 2, 1]),
                    axis=mybir.AxisListType.X,
                    op=mybir.AluOpType.add,
                )
                # accumulate progressively so only the last add is in the tail
                if k == 0:
                    nc.vector.tensor_copy(r[:], rk[0][:])
                else:
                    nc.vector.tensor_tensor(
                        out=r[:], in0=r[:], in1=rk[k][:], op=mybir.AluOpType.add
                    )
            # fold partition halves + bias
            nc.sync.dma_start(out=hi[:], in_=r[RPC:P, :])
            nc.vector.tensor_tensor(
                out=out_sb[:], in0=r[0:RPC, :], in1=hi[:], op=mybir.AluOpType.add
            )
            nc.vector.tensor_tensor(
                out=out_sb[:], in0=out_sb[:], in1=bias_sb[:], op=mybir.AluOpType.add
            )
            nc.sync.dma_start(out=out_d[:], in_=out_sb[:])
    nc.compile()
    return nc


def _host_layouts(ids_shard: np.ndarray):
    """ids_shard [RPC, S] int32 -> (lo2 [128, 256, 2] f32, idx16 [128, NI//16] i16)."""
    ids_nat = (
        ids_shard.reshape(RPC, 2, S // 2).transpose(1, 0, 2).reshape(P, NI // P)
    )
    lo2 = np.repeat((ids_nat & 1).astype(np.float32)[:, :, None], C, axis=2)
    idh = (ids_nat >> 1).astype(np.int16)
    cols = []
    off = 0
    for n in CHUNK_SIZES:
        jc = n // P
        sub = idh[:, off : off + jc]  # [128, jc]
        a = sub.reshape(8, 16, jc)  # (p//16, p%16, jj)
        t = a.transpose(1, 2, 0).reshape(16, jc * 8)  # [16, n//16]
        cols.append(np.tile(t, (8, 1)))  # replicate to 128 partitions
        off += jc
    idx16 = np.concatenate(cols, axis=1)  # [128, NI//16]
    return np.ascontiguousarray(lo2), np.ascontiguousarray(idx16)


def _build_table(W: np.ndarray) -> np.ndarray:
    Wt = np.zeros((2 * QROWS, 2), dtype=np.float32)
    Wt[:V] = W.astype(np.float32).T
    Wt[0] = 0.0  # pad token contributes nothing
    table = np.zeros((QROWS, 64), dtype=np.float32)
    table[:, 0:4] = Wt.reshape(QROWS, 4)
    return table


def kernel(input_ids: np.ndarray, W: np.ndarray, b: np.ndarray) -> np.ndarray:
    global LAST_EXEC_TIME_NS
    ids = np.ascontiguousarray(np.asarray(input_ids, dtype=np.int32))
    table = _build_table(np.asarray(W, dtype=np.float32))
    bias = np.ascontiguousarray(
        np.tile(np.asarray(b, dtype=np.float32)[None, :], (RPC, 1))
    )

    if "nc" not in _cache:
        _cache["nc"] = _build()
    nc = _cache["nc"]

    in_maps = []
    for c in range(NCORES):
        lo2, idx16 = _host_layouts(ids[c * RPC : (c + 1) * RPC])
        in_maps.append({"lo2": lo2, "idx16": idx16, "table": table, "bias": bias})

    trace = bool(int(os.environ.get("BOW_TRACE", "0")))
    res = run_bass_kernel_spmd(nc, in_maps, list(range(NCORES)), trace=trace)
    LAST_EXEC_TIME_NS = res.exec_time_ns

    out = np.concatenate([res.results[i]["out"] for i in range(NCORES)], axis=0)
    return np.ascontiguousarray(out.astype(np.float32))


# revision 14
# speedup vs baseline: 1.4107x; 1.4107x over previous
"""Bag-of-words classifier kernel for Trainium2 (8 NeuronCores, data-parallel).

Math: logits[b, c] = sum_s [ids[b,s] != 0] * W[c, ids[b,s]] + b[c].

Per core (64 rows, 32768 tokens), the gather runs on the Ant dma_gather
SWDGE instruction with a radix-2 table:
  - table rows (256B stride) hold vocab pairs {2q, 2q+1} as 4 f32:
    [W0[2q], W1[2q], W0[2q+1], W1[2q+1]]; row 0 slots 0:2 zeroed (pad mask).
  - gather index = id >> 1 (max 25128, fits int16), elem_size = 4 f32 (16B).
  - DVE selects the pair by lo = id & 1, reduces over tokens,
    folds partition halves, adds bias.

Token order i <-> (p = i % 128, j = i // 128 within chunk); partition p < 64
holds row p tokens [0,256), p >= 64 holds row p-64 tokens [256,512).

v2: idx>>1 (int16) and lo-bit selection masks are precomputed on host, so
gathers issue as soon as one input DMA lands; all chunks share one
num_idxs register (avoids NX WAR stalls between gather dispatches); one
gather per SWDGE queue, emitted back-to-back.
"""

import os

import numpy as np

import concourse.bass as bass
import concourse.tile as tile
from concourse import bacc, mybir
from concourse.bass_utils import run_bass_kernel_spmd

B, S, V, C = 512, 512, 50257, 2
NCORES = 8
RPC = B // NCORES  # rows per core = 64
P = 128
NI = RPC * S  # gathers per core = 32768
QROWS = (V + 1) // 2 + 1  # 25130 radix-2 table rows (incl. pad row for odd V)

NQUEUES = int(os.environ.get("BOW_NQUEUES", "4"))
SCRATCH = int(os.environ.get("BOW_SCRATCH", "131072"))
# (queue, idxs) per wave; one gather per queue per wave, so wave N's SDMA
# drain overlaps wave N+1's descriptor generation and the final drain tail
# is only the (small) last wave. Queue 0 measures ~3-4us slower per
# instruction than queues 1-3, so it gets smaller chunks.
_WAVE_SPEC = os.environ.get(
    "BOW_WAVESPEC",
    "1:5632,2:5632,3:5632,0:5120;1:2176,2:2176,3:2176,0:1280;1:896,2:896,3:896,0:256",
)
WAVE_CHUNKS = [
    [(int(q), int(n)) for q, n in (c.split(":") for c in w.split(","))]
    for w in _WAVE_SPEC.split(";")
]
CHUNK_QUEUES = [q for w in WAVE_CHUNKS for q, _ in w]
CHUNK_SIZES = [n for w in WAVE_CHUNKS for _, n in w]
assert sum(CHUNK_SIZES) == NI, sum(CHUNK_SIZES)
assert all(n % 128 == 0 for n in CHUNK_SIZES)
NCHUNKS = len(CHUNK_SIZES)

LAST_EXEC_TIME_NS = None

_cache = {}


def _raw_dma_gather(
    nc, out_ap, in_ap, idxs_ap, num_idxs_reg, num_idxs, elem_size, elem_step,
    queue_num=0,
):
    """bass dma_gather without the elem_size*dtype%256 assert (ucode handles
    small elem_size with a 256B-multiple row stride; HW-verified)."""
    from concourse.bass import exact_div

    g = nc.gpsimd
    assert idxs_ap.dtype == mybir.dt.int16
    stride_bytes = elem_step * mybir.dt.size(in_ap.dtype)
    stride_bytes_256 = exact_div(stride_bytes, 256)
    _in_ap = g.lower_ap_dma(in_ap, for_custom_bir_dma=True)
    _idxs_ap = g.lower_ap(idxs_ap)
    _out_ap = g.lower_ap(out_ap)
    return g.add_instruction(
        mybir.InstDMAGatherAnt(
            name=nc.get_next_instruction_name(),
            ins=[*_in_ap, _idxs_ap, g.lower_val_access(num_idxs_reg)],
            outs=[_out_ap],
            transpose=False,
            num_idxs=num_idxs,
            elem_size=elem_size,
            stride_bytes_256=stride_bytes_256,
            gen_mode=0,
            single_packet=False,
            queue_num=queue_num,
            sbuf_tokens_per_rank=0,
            sbuf_free_dim_per_rank=0,
            sbuf_free_dim_pad_per_rank=0,
            sbuf_byte_offset=0,
        )
    )


def _build() -> bass.Bass:
    nc = bacc.Bacc(
        "TRN2",
        target_bir_lowering=False,
        debug=False,
        num_devices=NCORES,
        num_swdge_queues=NQUEUES,
        dynamic_dma_scratch_size=SCRATCH,
    )
    # wrapped+replicated (id >> 1) in gather order, int16, host-prepared
    idx16_d = nc.dram_tensor("idx16", [P, NI // 16], mybir.dt.int16, kind="ExternalInput")
    # lo2[p, j, c] = float(ids[p, j] & 1) replicated over c, host-prepared
    lo2_d = nc.dram_tensor("lo2", [P, NI // P, C], mybir.dt.float32, kind="ExternalInput")
    tab_d = nc.dram_tensor("table", [QROWS, 64], mybir.dt.float32, kind="ExternalInput")
    bias_d = nc.dram_tensor("bias", [RPC, C], mybir.dt.float32, kind="ExternalInput")
    out_d = nc.dram_tensor("out", [RPC, C], mybir.dt.float32, kind="ExternalOutput")

    sizes = CHUNK_SIZES
    offs = [sum(sizes[:k]) for k in range(len(sizes))]  # token offsets (idxs)

    with tile.TileContext(nc) as tc:
        with tc.tile_pool(name="sbuf", bufs=1) as pool:
            lo2 = pool.tile([P, NI // P, C], mybir.dt.float32)
            r = pool.tile([P, C], mybir.dt.float32)
            hi = pool.tile([RPC, C], mybir.dt.float32)
            bias_sb = pool.tile([RPC, C], mybir.dt.float32)
            out_sb = pool.tile([RPC, C], mybir.dt.float32)

            # warmup gathers measured as net regressions twice (sem-lane
            # coupling stalls the real gathers); leave disabled
            if int(os.environ.get("BOW_WARMUP", "0")):
                widx = pool.tile([P, 8], mybir.dt.int16)
                wg = pool.tile([P, 1, 4], mybir.dt.float32)
                wjunk = pool.tile([P, 1, 4], mybir.dt.float32)
                nc.gpsimd.memset(widx[:], 0)
                wreg = nc.gpsimd.to_reg(128)
                _raw_dma_gather(nc, wg[:], tab_d[:, 0:4], widx[:], wreg, 128, 4,
                                64, queue_num=1)
                # consume wg promptly so its DMA-sem lane recycles cleanly
                nc.vector.tensor_copy(wjunk[:], wg[:])

            idx16_k = [
                pool.tile([P, n // 16], mybir.dt.int16, name=f"idx16_{k}")
                for k, n in enumerate(sizes)
            ]
            g_k = [
                pool.tile([P, n // P, 4], mybir.dt.float32, name=f"g{k}")
                for k, n in enumerate(sizes)
            ]
            vals_k = [
                pool.tile([P, n // P, C], mybir.dt.float32, name=f"vals{k}")
                for k, n in enumerate(sizes)
            ]
            rk = [
                pool.tile([P, C], mybir.dt.float32, name=f"rk{k}")
                for k in range(len(sizes))
            ]

            # wave-1 idx slices ride Sync, the rest ride Scalar, so the
            # first gathers are not queued behind later-wave input DMAs
            nwave1 = len(WAVE_CHUNKS[0])
            for k, n in enumerate(sizes):
                ws = slice(offs[k] // 16, (offs[k] + n) // 16)
                eng = nc.sync if k < nwave1 else nc.scalar
                eng.dma_start(out=idx16_k[k][:], in_=idx16_d[:, ws])
            nc.scalar.dma_start(out=lo2[:], in_=lo2_d[:])
            nc.scalar.dma_start(out=bias_sb[:], in_=bias_d[:])

            # one register per distinct num_idxs value, shared across chunks
            # (avoids WAR stalls at the NX between gather dispatches)
            regs = {}
            for n in set(sizes):
                regs[n] = nc.gpsimd.to_reg(n)

            for k, n in enumerate(sizes):
                _raw_dma_gather(
                    nc,
                    g_k[k][:],
                    tab_d[:, 0:4],
                    idx16_k[k][:],
                    regs[n],
                    n,
                    4,
                    64,
                    queue_num=CHUNK_QUEUES[k],
                )

            for k, n in enumerate(sizes):
                jc = n // P
                js = slice(offs[k] // P, offs[k] // P + jc)
                # vals = g02 + lo2 * (g24 - g02)
                nc.vector.tensor_tensor(
                    out=vals_k[k][:],
                    in0=g_k[k][:, :, 2:4],
                    in1=g_k[k][:, :, 0:2],
                    op=mybir.AluOpType.subtract,
                )
                nc.vector.tensor_tensor(
                    out=vals_k[k][:],
                    in0=vals_k[k][:],
                    in1=lo2[:, js, :],
                    op=mybir.AluOpType.mult,
                )
                nc.vector.tensor_tensor(
                    out=vals_k[k][:],
                    in0=vals_k[k][:],
                    in1=g_k[k][:, :, 0:2],
                    op=mybir.AluOpType.add,
                )
                # rk[p, c] = sum_j vals[p, j, c]
                nc.vector.tensor_reduce(
                    out=rk[k][:],
                    in_=vals_k[k][:].transpose([0, 2, 1]),
                    axis=mybir.AxisListType.X,
                    op=mybir.AluOpType.add,
                )
                # accumulate progressively so only the last add is in the tail
                if k == 0:
                    nc.vector.tensor_copy(r[:], rk[0][:])
                else:
                    nc.vector.tensor_tensor(
                        out=r[:], in0=r[:], in1=rk[k][:], op=mybir.AluOpType.add
                    )
            # fold partition halves + bias
            nc.sync.dma_start(out=hi[:], in_=r[RPC:P, :])
            nc.vector.tensor_tensor(
                out=out_sb[:], in0=r[0:RPC, :], in1=hi[:], op=mybir.AluOpType.add
            )
            nc.vector.tensor_tensor(
                out=out_sb[:], in0=out_sb[:], in1=bias_sb[:], op=mybir.AluOpType.add
            )
            nc.sync.dma_start(out=out_d[:], in_=out_sb[:])
    nc.compile()
    return nc


def _host_layouts(ids_shard: np.ndarray):
    """ids_shard [RPC, S] int32 -> (lo2 [128, 256, 2] f32, idx16 [128, NI//16] i16)."""
    ids_nat = (
        ids_shard.reshape(RPC, 2, S // 2).transpose(1, 0, 2).reshape(P, NI // P)
    )
    lo2 = np.repeat((ids_nat & 1).astype(np.float32)[:, :, None], C, axis=2)
    idh = (ids_nat >> 1).astype(np.int16)
    cols = []
    off = 0
    for n in CHUNK_SIZES:
        jc = n // P
        sub = idh[:, off : off + jc]  # [128, jc]
        a = sub.reshape(8, 16, jc)  # (p//16, p%16, jj)
        t = a.transpose(1, 2, 0).reshape(16, jc * 8)  # [16, n//16]
        cols.append(np.tile(t, (8, 1)))  # replicate to 128 partitions
        off += jc
    idx16 = np.concatenate(cols, axis=1)  # [128, NI//16]
    return np.ascontiguousarray(lo2), np.ascontiguousarray(idx16)


def _build_table(W: np.ndarray) -> np.ndarray:
    Wt = np.zeros((2 * QROWS, 2), dtype=np.float32)
    Wt[:V] = W.astype(np.float32).T
    Wt[0] = 0.0  # pad token contributes nothing
    table = np.zeros((QROWS, 64), dtype=np.float32)
    table[:, 0:4] = Wt.reshape(QROWS, 4)
    return table


def kernel(input_ids: np.ndarray, W: np.ndarray, b: np.ndarray) -> np.ndarray:
    global LAST_EXEC_TIME_NS
    ids = np.ascontiguousarray(np.asarray(input_ids, dtype=np.int32))
    table = _build_table(np.asarray(W, dtype=np.float32))
    bias = np.ascontiguousarray(
        np.tile(np.asarray(b, dtype=np.float32)[None, :], (RPC, 1))
    )

    if "nc" not in _cache:
        _cache["nc"] = _build()
    nc = _cache["nc"]

    in_maps = []
    for c in range(NCORES):
        lo2, idx16 = _host_layouts(ids[c * RPC : (c + 1) * RPC])
        in_maps.append({"lo2": lo2, "idx16": idx16, "table": table, "bias": bias})

    trace = bool(int(os.environ.get("BOW_TRACE", "0")))
    res = run_bass_kernel_spmd(nc, in_maps, list(range(NCORES)), trace=trace)
    LAST_EXEC_TIME_NS = res.exec_time_ns

    out = np.concatenate([res.results[i]["out"] for i in range(NCORES)], axis=0)
    return np.ascontiguousarray(out.astype(np.float32))


# revision 17
# speedup vs baseline: 1.4805x; 1.0495x over previous
"""Bag-of-words classifier kernel for Trainium2 (8 NeuronCores, data-parallel).

Math: logits[b, c] = sum_s [ids[b,s] != 0] * W[c, ids[b,s]] + b[c].

Per core (64 rows, 32768 tokens), the gather runs on the Ant dma_gather
SWDGE instruction with a radix-2 table:
  - table rows (256B stride) hold vocab pairs {2q, 2q+1} as 4 f32:
    [W0[2q], W1[2q], W0[2q+1], W1[2q+1]]; row 0 slots 0:2 zeroed (pad mask).
  - gather index = id >> 1 (max 25128, fits int16), elem_size = 4 f32 (16B).
  - DVE selects the pair by lo = id & 1, reduces over tokens,
    folds partition halves, adds bias.

Token order i <-> (p = i % 128, j = i // 128 within chunk); partition p < 64
holds row p tokens [0,256), p >= 64 holds row p-64 tokens [256,512).

v2: idx>>1 (int16) and lo-bit selection masks are precomputed on host, so
gathers issue as soon as one input DMA lands; all chunks share one
num_idxs register (avoids NX WAR stalls between gather dispatches); one
gather per SWDGE queue, emitted back-to-back.
"""

import os

import numpy as np

import concourse.bass as bass
import concourse.tile as tile
from concourse import bacc, mybir
from concourse.bass_utils import run_bass_kernel_spmd

B, S, V, C = 512, 512, 50257, 2
NCORES = 8
RPC = B // NCORES  # rows per core = 64
P = 128
NI = RPC * S  # gathers per core = 32768
QROWS = (V + 1) // 2 + 1  # 25130 radix-2 table rows (incl. pad row for odd V)

NQUEUES = int(os.environ.get("BOW_NQUEUES", "4"))
SCRATCH = int(os.environ.get("BOW_SCRATCH", "32768"))
# (queue, idxs) per wave; one gather per queue per wave, so wave N's SDMA
# drain overlaps wave N+1's descriptor generation and the final drain tail
# is only the (small) last wave. Queue 0 measures ~3-4us slower per
# instruction than queues 1-3, so it gets smaller chunks.
_WAVE_SPEC = os.environ.get(
    "BOW_WAVESPEC",
    "1:5376,2:5376,3:5376,0:4608;1:2048,2:2048,3:2048,0:1920;1:1024,2:1024,3:1024,0:896",
)
WAVE_CHUNKS = [
    [(int(q), int(n)) for q, n in (c.split(":") for c in w.split(","))]
    for w in _WAVE_SPEC.split(";")
]
CHUNK_QUEUES = [q for w in WAVE_CHUNKS for q, _ in w]
CHUNK_SIZES = [n for w in WAVE_CHUNKS for _, n in w]
assert sum(CHUNK_SIZES) == NI, sum(CHUNK_SIZES)
assert all(n % 128 == 0 for n in CHUNK_SIZES)
NCHUNKS = len(CHUNK_SIZES)

LAST_EXEC_TIME_NS = None

_cache = {}


def _raw_dma_gather(
    nc, out_ap, in_ap, idxs_ap, num_idxs_reg, num_idxs, elem_size, elem_step,
    queue_num=0,
):
    """bass dma_gather without the elem_size*dtype%256 assert (ucode handles
    small elem_size with a 256B-multiple row stride; HW-verified)."""
    from concourse.bass import exact_div

    g = nc.gpsimd
    assert idxs_ap.dtype == mybir.dt.int16
    stride_bytes = elem_step * mybir.dt.size(in_ap.dtype)
    stride_bytes_256 = exact_div(stride_bytes, 256)
    _in_ap = g.lower_ap_dma(in_ap, for_custom_bir_dma=True)
    _idxs_ap = g.lower_ap(idxs_ap)
    _out_ap = g.lower_ap(out_ap)
    return g.add_instruction(
        mybir.InstDMAGatherAnt(
            name=nc.get_next_instruction_name(),
            ins=[*_in_ap, _idxs_ap, g.lower_val_access(num_idxs_reg)],
            outs=[_out_ap],
            transpose=False,
            num_idxs=num_idxs,
            elem_size=elem_size,
            stride_bytes_256=stride_bytes_256,
            gen_mode=0,
            single_packet=False,
            queue_num=queue_num,
            sbuf_tokens_per_rank=0,
            sbuf_free_dim_per_rank=0,
            sbuf_free_dim_pad_per_rank=0,
            sbuf_byte_offset=0,
        )
    )


def _build() -> bass.Bass:
    nc = bacc.Bacc(
        "TRN2",
        target_bir_lowering=False,
        debug=False,
        num_devices=NCORES,
        num_swdge_queues=NQUEUES,
        dynamic_dma_scratch_size=SCRATCH,
    )
    # wrapped+replicated (id >> 1) in gather order, int16, host-prepared
    idx16_d = nc.dram_tensor("idx16", [P, NI // 16], mybir.dt.int16, kind="ExternalInput")
    # lo2[p, j, c] = float(ids[p, j] & 1) replicated over c, host-prepared
    lo2_d = nc.dram_tensor("lo2", [P, NI // P, C], mybir.dt.float32, kind="ExternalInput")
    tab_d = nc.dram_tensor("table", [QROWS, 64], mybir.dt.float32, kind="ExternalInput")
    bias_d = nc.dram_tensor("bias", [RPC, C], mybir.dt.float32, kind="ExternalInput")
    out_d = nc.dram_tensor("out", [RPC, C], mybir.dt.float32, kind="ExternalOutput")

    sizes = CHUNK_SIZES
    offs = [sum(sizes[:k]) for k in range(len(sizes))]  # token offsets (idxs)

    with tile.TileContext(nc) as tc:
        with tc.tile_pool(name="sbuf", bufs=1) as pool:
            if int(os.environ.get("BOW_EARLY_LIB", "1")):
                # load the GPSIMD library holding InstDMAGatherAnt up front so
                # the ~6us Q7 IRAM load overlaps the input DMAs instead of
                # gating the first gather
                from concourse import library_config

                nc.gpsimd.load_library(library_config.mlp)

            lo2 = pool.tile([P, NI // P, C], mybir.dt.float32)
            r = pool.tile([P, C], mybir.dt.float32)
            hi = pool.tile([RPC, C], mybir.dt.float32)
            bias_sb = pool.tile([RPC, C], mybir.dt.float32)
            out_sb = pool.tile([RPC, C], mybir.dt.float32)

            # warmup gathers measured as net regressions twice (sem-lane
            # coupling stalls the real gathers); leave disabled
            if int(os.environ.get("BOW_WARMUP", "0")):
                widx = pool.tile([P, 8], mybir.dt.int16)
                wg = pool.tile([P, 1, 4], mybir.dt.float32)
                wjunk = pool.tile([P, 1, 4], mybir.dt.float32)
                nc.gpsimd.memset(widx[:], 0)
                wreg = nc.gpsimd.to_reg(128)
                _raw_dma_gather(nc, wg[:], tab_d[:, 0:4], widx[:], wreg, 128, 4,
                                64, queue_num=1)
                # consume wg promptly so its DMA-sem lane recycles cleanly
                nc.vector.tensor_copy(wjunk[:], wg[:])

            idx16_k = [
                pool.tile([P, n // 16], mybir.dt.int16, name=f"idx16_{k}")
                for k, n in enumerate(sizes)
            ]
            g_k = [
                pool.tile([P, n // P, 4], mybir.dt.float32, name=f"g{k}")
                for k, n in enumerate(sizes)
            ]
            vals_k = [
                pool.tile([P, n // P, C], mybir.dt.float32, name=f"vals{k}")
                for k, n in enumerate(sizes)
            ]
            rk = [
                pool.tile([P, C], mybir.dt.float32, name=f"rk{k}")
                for k in range(len(sizes))
            ]

            # wave-1 idx slices ride Sync, the rest ride Scalar, so the
            # first gathers are not queued behind later-wave input DMAs
            nwave1 = len(WAVE_CHUNKS[0])
            for k, n in enumerate(sizes):
                ws = slice(offs[k] // 16, (offs[k] + n) // 16)
                eng = nc.sync if k < nwave1 else nc.scalar
                eng.dma_start(out=idx16_k[k][:], in_=idx16_d[:, ws])
            nc.scalar.dma_start(out=lo2[:], in_=lo2_d[:])
            nc.scalar.dma_start(out=bias_sb[:], in_=bias_d[:])

            # one register per distinct num_idxs value, shared across chunks
            # (avoids WAR stalls at the NX between gather dispatches)
            regs = {}
            for n in set(sizes):
                regs[n] = nc.gpsimd.to_reg(n)

            for k, n in enumerate(sizes):
                _raw_dma_gather(
                    nc,
                    g_k[k][:],
                    tab_d[:, 0:4],
                    idx16_k[k][:],
                    regs[n],
                    n,
                    4,
                    64,
                    queue_num=CHUNK_QUEUES[k],
                )

            for k, n in enumerate(sizes):
                jc = n // P
                js = slice(offs[k] // P, offs[k] // P + jc)
                # vals = g02 + lo2 * (g24 - g02)
                nc.vector.tensor_tensor(
                    out=vals_k[k][:],
                    in0=g_k[k][:, :, 2:4],
                    in1=g_k[k][:, :, 0:2],
                    op=mybir.AluOpType.subtract,
                )
                nc.vector.tensor_tensor(
                    out=vals_k[k][:],
                    in0=vals_k[k][:],
                    in1=lo2[:, js, :],
                    op=mybir.AluOpType.mult,
                )
                nc.vector.tensor_tensor(
                    out=vals_k[k][:],
                    in0=vals_k[k][:],
                    in1=g_k[k][:, :, 0:2],
                    op=mybir.AluOpType.add,
                )
                # rk[p, c] = sum_j vals[p, j, c]
                nc.vector.tensor_reduce(
                    out=rk[k][:],
                    in_=vals_k[k][:].transpose([0, 2, 1]),
                    axis=mybir.AxisListType.X,
                    op=mybir.AluOpType.add,
                )
                # accumulate progressively so only the last add is in the tail
                if k == 0:
                    nc.vector.tensor_copy(r[:], rk[0][:])
                else:
                    nc.vector.tensor_tensor(
                        out=r[:], in0=r[:], in1=rk[k][:], op=mybir.AluOpType.add
                    )
            # fold partition halves + bias
            nc.sync.dma_start(out=hi[:], in_=r[RPC:P, :])
            nc.vector.tensor_tensor(
                out=out_sb[:], in0=r[0:RPC, :], in1=hi[:], op=mybir.AluOpType.add
            )
            nc.vector.tensor_tensor(
                out=out_sb[:], in0=out_sb[:], in1=bias_sb[:], op=mybir.AluOpType.add
            )
            nc.sync.dma_start(out=out_d[:], in_=out_sb[:])
    nc.compile()
    return nc


def _host_layouts(ids_shard: np.ndarray):
    """ids_shard [RPC, S] int32 -> (lo2 [128, 256, 2] f32, idx16 [128, NI//16] i16)."""
    ids_nat = (
        ids_shard.reshape(RPC, 2, S // 2).transpose(1, 0, 2).reshape(P, NI // P)
    )
    lo2 = np.repeat((ids_nat & 1).astype(np.float32)[:, :, None], C, axis=2)
    idh = (ids_nat >> 1).astype(np.int16)
    cols = []
    off = 0
    for n in CHUNK_SIZES:
        jc = n // P
        sub = idh[:, off : off + jc]  # [128, jc]
        a = sub.reshape(8, 16, jc)  # (p//16, p%16, jj)
        t = a.transpose(1, 2, 0).reshape(16, jc * 8)  # [16, n//16]
        cols.append(np.tile(t, (8, 1)))  # replicate to 128 partitions
        off += jc
    idx16 = np.concatenate(cols, axis=1)  # [128, NI//16]
    return np.ascontiguousarray(lo2), np.ascontiguousarray(idx16)


def _build_table(W: np.ndarray) -> np.ndarray:
    Wt = np.zeros((2 * QROWS, 2), dtype=np.float32)
    Wt[:V] = W.astype(np.float32).T
    Wt[0] = 0.0  # pad token contributes nothing
    table = np.zeros((QROWS, 64), dtype=np.float32)
    table[:, 0:4] = Wt.reshape(QROWS, 4)
    return table


def kernel(input_ids: np.ndarray, W: np.ndarray, b: np.ndarray) -> np.ndarray:
    global LAST_EXEC_TIME_NS
    ids = np.ascontiguousarray(np.asarray(input_ids, dtype=np.int32))
    table = _build_table(np.asarray(W, dtype=np.float32))
    bias = np.ascontiguousarray(
        np.tile(np.asarray(b, dtype=np.float32)[None, :], (RPC, 1))
    )

    if "nc" not in _cache:
        _cache["nc"] = _build()
    nc = _cache["nc"]

    in_maps = []
    for c in range(NCORES):
        lo2, idx16 = _host_layouts(ids[c * RPC : (c + 1) * RPC])
        in_maps.append({"lo2": lo2, "idx16": idx16, "table": table, "bias": bias})

    trace = bool(int(os.environ.get("BOW_TRACE", "0")))
    res = run_bass_kernel_spmd(nc, in_maps, list(range(NCORES)), trace=trace)
    LAST_EXEC_TIME_NS = res.exec_time_ns

    out = np.concatenate([res.results[i]["out"] for i in range(NCORES)], axis=0)
    return np.ascontiguousarray(out.astype(np.float32))
